# Initial kernel scaffold
#
"""Trainium2 Bass kernel for nn_AttenModule (B=64, N=1024, M=80, C1=288, D=256).

Math notes (derived from the reference):
  score[b,n,m] = (oa@w_o)[b,n] + (lang@w_l)[b,m] + ba, softmax over m.
  The (oa@w_o)[b,n] and ba terms are constant along m, so they cancel in the
  softmax -> att[b,n,:] == softmax_m(mask(lang[b]@w_l)) is independent of n,
  and att_feat[b,:] = sum_m att[b,m]*lang[b,m,:] is a per-batch vector.
  Hence the entire W1/W2/w_o branch is dead.

  Remaining per-row work (row = (b,n)):
    osc = relu(x @ W3 + b3) @ W4 + b4            # x = object_feat row (288,)
    v   = relu(osc * att_feat[b])                # (256,)
    out = (v @ Ws) / max(||v||, 1e-12) + bs      # scalar

Device layout: feature-on-partition (transposed activations).  Per core
(8 cores, data-parallel over B): 8 batches = 8192 rows, row-tiles of 512.
    h^T[o]  (128,512) = sum_c W3[c,o128].T @ x^T[c]      (3 K-chunks: 128/128/32)
    x^T[o]  = relu(ps + b3) ... relu(ps*af + b4*af) via ACT per-partition scale
    dot(1,512) = Ws^T @ x^T ;  ss(1,512) = ones^T @ (x^T)^2   (PE matvecs)
    out = dot / sqrt(ss + 1e-24) + bs
"""

import numpy as np

import concourse.bass as bass
import concourse.tile as tile
from concourse import mybir
from concourse.bass_utils import run_bass_kernel_spmd

B, N, M = 64, 1024, 80
C1, D = 288, 256
NCORES = 8
BPC = B // NCORES          # batches per core
R = BPC * N                # rows per core
TILE = 512
NT = R // TILE             # row tiles per core
F32 = mybir.dt.float32

# matmul input dtype: float32r streams 1 row/cycle (vs 4 for float32) at N>=256
MM_DT = mybir.dt.float32r


def _mm(x):
    return x.bitcast(MM_DT) if MM_DT != F32 else x


def _build_nc():
    nc = bass.Bass("TRN2", target_bir_lowering=False, debug=False)

    xt_d = nc.dram_tensor("xt", [C1, R], F32, kind="ExternalInput").ap()
    langm_d = nc.dram_tensor("langm", [M, BPC, D], F32, kind="ExternalInput").ap()
    langt2_d = nc.dram_tensor("langt2", [D, BPC * M], F32, kind="ExternalInput").ap()
    maskt_d = nc.dram_tensor("maskt", [M, BPC], F32, kind="ExternalInput").ap()
    w3_d = nc.dram_tensor("w3", [C1, D], F32, kind="ExternalInput").ap()
    w4_d = nc.dram_tensor("w4", [D, D], F32, kind="ExternalInput").ap()
    b3_d = nc.dram_tensor("b3", [128, 2], F32, kind="ExternalInput").ap()
    b4_d = nc.dram_tensor("b4", [128, 2], F32, kind="ExternalInput").ap()
    ws_d = nc.dram_tensor("ws", [128, 2], F32, kind="ExternalInput").ap()
    wl_d = nc.dram_tensor("wl", [128, 2], F32, kind="ExternalInput").ap()
    bs_d = nc.dram_tensor("bs", [1, 1], F32, kind="ExternalInput").ap()
    out_d = nc.dram_tensor("out", [1, R], F32, kind="ExternalOutput").ap()

    AF = mybir.ActivationFunctionType

    with tile.TileContext(nc) as tc:
        with tc.tile_pool(name="const", bufs=1) as cp:
            w3a = cp.tile([128, D], F32)
            w3b = cp.tile([128, D], F32)
            w3c = cp.tile([32, D], F32)
            nc.sync.dma_start(out=w3a, in_=w3_d[0:128, :])
            nc.sync.dma_start(out=w3b, in_=w3_d[128:256, :])
            nc.sync.dma_start(out=w3c, in_=w3_d[256:288, :])
            w4a = cp.tile([128, D], F32)
            w4b = cp.tile([128, D], F32)
            nc.sync.dma_start(out=w4a, in_=w4_d[0:128, :])
            nc.sync.dma_start(out=w4b, in_=w4_d[128:256, :])
            b3s = cp.tile([128, 2], F32)
            b4s = cp.tile([128, 2], F32)
            wss = cp.tile([128, 2], F32)
            wls = cp.tile([128, 2], F32)
            bss = cp.tile([1, 1], F32)
            nc.sync.dma_start(out=b3s, in_=b3_d)
            nc.sync.dma_start(out=b4s, in_=b4_d)
            nc.sync.dma_start(out=wss, in_=ws_d)
            nc.sync.dma_start(out=wls, in_=wl_d)
            nc.sync.dma_start(out=bss, in_=bs_d)
            langm = cp.tile([M, BPC, D], F32)
            nc.sync.dma_start(out=langm, in_=langm_d)
            langt2 = cp.tile([128, 2, BPC, M], F32)
            lt2v = langt2_d.rearrange("(c p) (b m) -> c p b m", c=2, b=BPC)
            nc.sync.dma_start(out=langt2[:, 0], in_=lt2v[0])
            nc.sync.dma_start(out=langt2[:, 1], in_=lt2v[1])
            maskt = cp.tile([M, BPC], F32)
            nc.sync.dma_start(out=maskt, in_=maskt_d)
            ones_m = cp.tile([M, 1], F32)
            nc.vector.memset(ones_m, 1.0)
            ones_1x128 = cp.tile([1, 128], F32)
            nc.vector.memset(ones_1x128, 1.0)
            ones_128 = cp.tile([128, 1], F32)
            nc.vector.memset(ones_128, 1.0)

            # ---------- attention branch (per-batch, tiny) ----------
            att_sb = tc.tile_pool(name="att_sb", bufs=1)
            with tc.tile_pool(name="att_ps", bufs=1, space="PSUM") as ap_ps, att_sb:
                ps_sl = ap_ps.tile([M, BPC], F32)
                for b in range(BPC):
                    for c in range(2):
                        nc.tensor.matmul(
                            ps_sl[:, b : b + 1],
                            _mm(langt2[:, c, b, :]),
                            _mm(wls[:, c : c + 1]),
                            start=(c == 0),
                            stop=(c == 1),
                        )
                e_sb = att_sb.tile([M, BPC], F32)
                nc.scalar.activation(e_sb, ps_sl, AF.Exp)
                em_sb = att_sb.tile([M, BPC], F32)
                nc.vector.tensor_mul(em_sb, e_sb, maskt)
                # denom (1, BPC) = ones^T @ em
                ps_dn = ap_ps.tile([1, BPC], F32)
                nc.tensor.matmul(ps_dn, _mm(ones_m), _mm(em_sb), start=True, stop=True)
                rd_sb = att_sb.tile([1, BPC], F32)
                nc.vector.reciprocal(rd_sb, ps_dn)
                # broadcast 1/denom to 128 partitions via K=1 matmul
                ps_rdb = ap_ps.tile([128, BPC], F32)
                nc.tensor.matmul(
                    ps_rdb, _mm(ones_1x128), _mm(rd_sb), start=True, stop=True
                )
                rdb_sb = att_sb.tile([128, BPC], F32)
                nc.vector.tensor_copy(rdb_sb, ps_rdb)
                # af_raw (128, 2, BPC) = lang[b].T @ em[:, b]
                ps_af = ap_ps.tile([128, 2, BPC], F32)
                for b in range(BPC):
                    for c in range(2):
                        nc.tensor.matmul(
                            ps_af[:, c, b : b + 1],
                            _mm(langm[:, b, c * 128 : (c + 1) * 128]),
                            _mm(em_sb[:, b : b + 1]),
                            start=True,
                            stop=True,
                        )
                af = cp.tile([128, 2, BPC], F32)
                for c in range(2):
                    nc.vector.tensor_mul(af[:, c, :], ps_af[:, c, :], rdb_sb)
                b4af = cp.tile([128, 2, BPC], F32)
                for b in range(BPC):
                    nc.vector.tensor_mul(b4af[:, :, b], af[:, :, b], b4s)

            # ---------- main loop ----------
            with (
                tc.tile_pool(name="xt", bufs=2) as xp,
                tc.tile_pool(name="work", bufs=3) as wp,
                tc.tile_pool(name="ep", bufs=3) as epp,
                tc.tile_pool(name="outp", bufs=1) as op_,
                tc.tile_pool(name="ph", bufs=2, space="PSUM") as php,
                tc.tile_pool(name="po", bufs=2, space="PSUM") as pop,
                tc.tile_pool(name="pmv", bufs=2, space="PSUM") as pmv,
            ):
                out_sb = op_.tile([1, NT, TILE], F32)
                xtv = xt_d.rearrange("c (bb r) -> c bb r", bb=BPC)
                for bb in range(BPC):
                    # one batch = 1024 rows = 2 row-tiles; load x^T per batch
                    x0 = xp.tile([128, N], F32, tag="x0")
                    x1 = xp.tile([128, N], F32, tag="x1")
                    x2 = xp.tile([32, N], F32, tag="x2")
                    nc.sync.dma_start(out=x0, in_=xtv[0:128, bb])
                    nc.sync.dma_start(out=x1, in_=xtv[128:256, bb])
                    nc.sync.dma_start(out=x2, in_=xtv[256:288, bb])
                    for tt in range(2):
                        t = bb * 2 + tt
                        rs = slice(tt * TILE, (tt + 1) * TILE)
                        hs = []
                        for o in range(2):
                            os_ = slice(o * 128, (o + 1) * 128)
                            ph = php.tile([128, TILE], F32, tag="ph")
                            nc.tensor.matmul(
                                ph, _mm(w3a[:, os_]), _mm(x0[:, rs]),
                                start=True, stop=False,
                            )
                            nc.tensor.matmul(
                                ph, _mm(w3b[:, os_]), _mm(x1[:, rs]),
                                start=False, stop=False,
                            )
                            nc.tensor.matmul(
                                ph, _mm(w3c[:, os_]), _mm(x2[:, rs]),
                                start=False, stop=True,
                            )
                            h = wp.tile([128, TILE], F32, tag=f"h{o}")
                            nc.scalar.activation(
                                h, ph, AF.Relu, bias=b3s[:, o : o + 1], scale=1.0
                            )
                            hs.append(h)
                        xs = []
                        xqs = []
                        for o in range(2):
                            os_ = slice(o * 128, (o + 1) * 128)
                            po = pop.tile([128, TILE], F32, tag="po")
                            nc.tensor.matmul(
                                po, _mm(w4a[:, os_]), _mm(hs[0]),
                                start=True, stop=False,
                            )
                            nc.tensor.matmul(
                                po, _mm(w4b[:, os_]), _mm(hs[1]),
                                start=False, stop=True,
                            )
                            xv = wp.tile([128, TILE], F32, tag=f"xv{o}")
                            nc.scalar.activation(
                                xv, po, AF.Relu,
                                bias=b4af[:, o, bb : bb + 1],
                                scale=af[:, o, bb : bb + 1],
                            )
                            xq = wp.tile([128, TILE], F32, tag=f"xq{o}")
                            nc.vector.tensor_mul(xq, xv, xv)
                            xs.append(xv)
                            xqs.append(xq)
                        ps_dot = pmv.tile([1, TILE], F32, tag="dot")
                        ps_ss = pmv.tile([1, TILE], F32, tag="ss")
                        nc.tensor.matmul(
                            ps_dot, _mm(wss[:, 0:1]), _mm(xs[0]),
                            start=True, stop=False,
                        )
                        nc.tensor.matmul(
                            ps_dot, _mm(wss[:, 1:2]), _mm(xs[1]),
                            start=False, stop=True,
                        )
                        nc.tensor.matmul(
                            ps_ss, _mm(ones_128), _mm(xqs[0]),
                            start=True, stop=False,
                        )
                        nc.tensor.matmul(
                            ps_ss, _mm(ones_128), _mm(xqs[1]),
                            start=False, stop=True,
                        )
                        # out = dot / sqrt(ss + 1e-24) + bs
                        rt = epp.tile([1, TILE], F32, tag="rt")
                        nc.scalar.activation(rt, ps_ss, AF.Sqrt, bias=1e-24)
                        rc = epp.tile([1, TILE], F32, tag="rc")
                        nc.vector.reciprocal(rc, rt)
                        tm = epp.tile([1, TILE], F32, tag="tm")
                        nc.vector.tensor_mul(tm, ps_dot, rc)
                        nc.scalar.activation(
                            out_sb[:, t, :], tm, AF.Identity, bias=bss[0:1, 0:1]
                        )
                nc.sync.dma_start(out=out_d, in_=out_sb.rearrange("p t r -> p (t r)"))
    return nc


_NC_CACHE = {}


def _get_nc():
    if "nc" not in _NC_CACHE:
        _NC_CACHE["nc"] = _build_nc()
    return _NC_CACHE["nc"]


def kernel(**inputs) -> np.ndarray:
    object_feat = np.ascontiguousarray(np.asarray(inputs["object_feat"], np.float32))
    lang_feat = np.ascontiguousarray(np.asarray(inputs["lang_feat"], np.float32))
    lang_mask = np.asarray(inputs["lang_mask"])
    W3 = np.ascontiguousarray(np.asarray(inputs["W3"], np.float32))
    W4 = np.ascontiguousarray(np.asarray(inputs["W4"], np.float32))
    b3 = np.asarray(inputs["b3"], np.float32)
    b4 = np.asarray(inputs["b4"], np.float32)
    Wa = np.asarray(inputs["Wa"], np.float32)
    Ws = np.asarray(inputs["Ws"], np.float32)
    bs = np.asarray(inputs["bs"], np.float32)

    b3h = np.ascontiguousarray(b3.reshape(2, 128).T)
    b4h = np.ascontiguousarray(b4.reshape(2, 128).T)
    wsh = np.ascontiguousarray(Ws[:, 0].reshape(2, 128).T)
    wlh = np.ascontiguousarray(Wa[D:, 0].reshape(2, 128).T)
    bsh = bs.reshape(1, 1)

    in_maps = []
    for i in range(NCORES):
        sl = slice(i * BPC, (i + 1) * BPC)
        of = object_feat[sl]                                   # (BPC, N, C1)
        lf = lang_feat[sl]                                     # (BPC, M, D)
        in_maps.append(
            {
                "xt": np.ascontiguousarray(of.reshape(R, C1).T),
                "langm": np.ascontiguousarray(lf.transpose(1, 0, 2)),
                "langt2": np.ascontiguousarray(
                    lf.transpose(2, 0, 1).reshape(D, BPC * M)
                ),
                "maskt": np.ascontiguousarray(lang_mask[sl].T.astype(np.float32)),
                "w3": W3,
                "w4": W4,
                "b3": b3h,
                "b4": b4h,
                "ws": wsh,
                "wl": wlh,
                "bs": bsh,
            }
        )

    nc = _get_nc()
    res = run_bass_kernel_spmd(nc, in_maps, core_ids=list(range(NCORES)))
    out = np.empty((B, 1, N), np.float32)
    for i in range(NCORES):
        out[i * BPC : (i + 1) * BPC, 0, :] = res.results[i]["out"].reshape(BPC, N)
    return out


# revision 42
# speedup vs baseline: 1.5496x; 1.5496x over previous
"""Trainium2 Bass kernel for nn_AttenModule (B=64, N=1024, M=80, C1=288, D=256).

Math notes (derived from the reference):
  score[b,n,m] = (oa@w_o)[b,n] + (lang@w_l)[b,m] + ba, softmax over m.
  The (oa@w_o)[b,n] and ba terms are constant along m, so they cancel in the
  softmax -> att[b,n,:] == softmax_m(mask(lang[b]@w_l)) is independent of n,
  and att_feat[b,:] = sum_m att[b,m]*lang[b,m,:] is a per-batch vector.
  Hence the entire W1/W2/w_o branch is dead.

  out = (v@Ws)/max(||v||,eps) + bs with v = relu(osc * af[b]) is
  scale-invariant in v (relu commutes with positive scales), so the softmax
  denominator only needs enough accuracy to keep fp16 ranges in check.

  Remaining per-row work (row = (b,n)):
    osc = relu(x @ W3 + b3) @ W4 + b4            # x = object_feat row (288,)
    v   = relu(osc * af[b])                      # af[b] = softmax(lang@w_l) @ lang
    out = (v @ Ws) / sqrt(||v||^2 + 1e-24) + bs

Device layout: feature-on-partition (transposed activations).  Per core
(8 cores, data-parallel over B): 8 batches = 8192 rows, row-tiles of 512.
All matmuls run in fp16: 11-bit mantissa (same as the PE's fp32r/tf32 mode)
but 1 cycle/row instead of 2 and half the HBM traffic.  PSUM accumulation
is fp32.  Biases and the attention scale vector stay fp32 (applied via the
ACT engine's per-partition scale/bias, which reads fp32 SBUF).
"""

import numpy as np

import concourse.bacc as bacc
import concourse.tile as tile
from concourse import mybir
from concourse.bass_utils import run_bass_kernel_spmd

B, N, M = 64, 1024, 80
C1, D = 288, 256
NCORES = 8
BPC = B // NCORES          # batches per core
R = BPC * N                # rows per core
TILE = 512
NT = R // TILE             # row tiles per core
F32 = mybir.dt.float32
F16 = mybir.dt.float16


def _build_nc():
    nc = bacc.Bacc("TRN2", target_bir_lowering=False, debug=False)

    # xt rows 0..287 = x^T; rows 288..319 duplicate rows 256..287 so the two
    # K=32 tail matmuls (one per out-chunk) can run in concurrent PE row-groups
    xt_d = nc.dram_tensor("xt", [C1 + 32, R], F16, kind="ExternalInput").ap()
    langm_d = nc.dram_tensor("langm", [M, BPC, D], F16, kind="ExternalInput").ap()
    # langt2 pre-arranged as [128, 2, BPC*M] on the host
    langt2_d = nc.dram_tensor("langt2", [128, 2 * BPC * M], F16, kind="ExternalInput").ap()
    # w3/w4 pre-arranged as [128, 2, D] (chunk-major)
    w3_d = nc.dram_tensor("w3", [128, 2 * D], F16, kind="ExternalInput").ap()
    # w3c2: rows 0-31 = W3[256:288, 0:128], rows 32-63 = W3[256:288, 128:256]
    w3c_d = nc.dram_tensor("w3c", [64, 128], F16, kind="ExternalInput").ap()
    w4_d = nc.dram_tensor("w4", [128, 2 * D], F16, kind="ExternalInput").ap()
    # packed fp32 consts: cols [b3(2) | b4(2) | bs(1) | maskt(8, rows 0-79)]
    cstf_d = nc.dram_tensor("cstf", [128, 13], F32, kind="ExternalInput").ap()
    # packed fp16 consts: cols [ws(2) | wl(2)]
    csth_d = nc.dram_tensor("csth", [128, 4], F16, kind="ExternalInput").ap()
    out_d = nc.dram_tensor("out", [1, R], F32, kind="ExternalOutput").ap()

    AF = mybir.ActivationFunctionType

    with tile.TileContext(nc) as tc:
        with tc.tile_pool(name="const", bufs=1) as cp:
            # consts go on the scalar-engine HWDGE queue so the first x-tile
            # loads (sync queue) aren't stuck behind them
            csth = cp.tile([128, 4], F16)
            nc.scalar.dma_start(out=csth, in_=csth_d)
            cstf = cp.tile([128, 13], F32)
            nc.scalar.dma_start(out=cstf, in_=cstf_d)
            langt2 = cp.tile([128, 2, BPC, M], F16)
            nc.scalar.dma_start(
                out=langt2, in_=langt2_d.rearrange("p (c bm) -> p c bm", c=2)
            )
            langm = cp.tile([M, BPC, D], F16)
            nc.scalar.dma_start(out=langm, in_=langm_d)
            w3t = cp.tile([128, 2, D], F16)
            nc.scalar.dma_start(out=w3t, in_=w3_d.rearrange("p (c d) -> p c d", c=2))
            w3c2 = cp.tile([64, 128], F16)
            nc.scalar.dma_start(out=w3c2, in_=w3c_d)
            w4t = cp.tile([128, 2, D], F16)
            nc.scalar.dma_start(out=w4t, in_=w4_d.rearrange("p (c d) -> p c d", c=2))
            wss = csth[:, 0:2]
            wls = csth[:, 2:4]
            b3s = cstf[:, 0:2]
            b4s = cstf[:, 2:4]
            bss = cstf[0:1, 4:5]
            maskt = cstf[0:M, 5:13]
            ones_m = cp.tile([M, 1], F16)
            nc.vector.memset(ones_m, 1.0)
            ones_1x128 = cp.tile([1, 128], F16)
            nc.vector.memset(ones_1x128, 1.0)
            ones_128 = cp.tile([128, 1], F16)
            nc.vector.memset(ones_128, 1.0)
            eps_sb = cp.tile([1, 1], F32)
            nc.vector.memset(eps_sb, 1e-24)

            # ---------- main loop (attention interleaved into it) ----------
            with (
                tc.tile_pool(name="xt", bufs=3) as xp,
                tc.tile_pool(name="work", bufs=4) as wp,
                tc.tile_pool(name="ep", bufs=3) as epp,
                tc.tile_pool(name="outp", bufs=1) as op_,
                tc.tile_pool(name="ph", bufs=1, space="PSUM") as php,
                tc.tile_pool(name="po", bufs=1, space="PSUM") as pop,
                tc.tile_pool(name="pmv", bufs=1, space="PSUM") as pmv,
            ):
                out_sb = op_.tile([1, NT, TILE], F32)
                xtv = xt_d.rearrange("c (bb r) -> c bb r", bb=BPC)

                # --- attention part A: scores, exp, mask, denom ---
                # (psum tiles share main-loop tags chosen to avoid slot cycles)
                ps_sl = pmv.tile([M, BPC], F32, tag="dot")
                for b in range(BPC):
                    for c in range(2):
                        nc.tensor.matmul(
                            ps_sl[:, b : b + 1],
                            langt2[:, c, b, :],
                            wls[:, c : c + 1],
                            start=(c == 0),
                            stop=(c == 1),
                        )
                e_sb = cp.tile([M, BPC], F32)
                nc.scalar.activation(e_sb, ps_sl, AF.Exp)
                em_sb = cp.tile([M, BPC], F16)
                nc.vector.tensor_mul(em_sb, e_sb, maskt)
                # denom per batch + reciprocal (the per-batch scale cancels in
                # the output; it only keeps fp16 magnitudes in range)
                ps_dn = pmv.tile([1, BPC], F32, tag="ss")
                nc.tensor.matmul(ps_dn, ones_m, em_sb, start=True, stop=True)
                rd32 = cp.tile([1, BPC], F32)
                nc.vector.reciprocal(rd32, ps_dn)
                rdf = cp.tile([1, BPC], F16)
                nc.vector.tensor_copy(rdf, rd32)
                af = cp.tile([128, 2, BPC], F32)
                b4af = cp.tile([128, 2, BPC], F32)

                def emit_attention_b():
                    # --- attention part B: af matmuls + scaling.  Emitted
                    # after tile 0's mains so its PE/DVE work fills the
                    # exp->em dependency bubble instead of stalling the head.
                    ps_rdb = php.tile([128, BPC], F32, tag="ph0")
                    nc.tensor.matmul(ps_rdb, ones_1x128, rdf, start=True, stop=True)
                    rdb = cp.tile([128, BPC], F32)
                    nc.vector.tensor_copy(rdb, ps_rdb)
                    ps_af = php.tile([128, 2, BPC], F32, tag="ph1")
                    for b in range(BPC):
                        for c in range(2):
                            nc.tensor.matmul(
                                ps_af[:, c, b : b + 1],
                                langm[:, b, c * 128 : (c + 1) * 128],
                                em_sb[:, b : b + 1],
                                start=True,
                                stop=True,
                            )
                    for c in range(2):
                        nc.vector.tensor_mul(af[:, c, :], ps_af[:, c, :], rdb)
                    for b in range(BPC):
                        nc.vector.tensor_mul(b4af[:, :, b], af[:, :, b], b4s)

                mv_ps = {}

                def emit_mv_epilogue(t, xs, xqs):
                    # matvecs per tile; epilogue batched over tile pairs so the
                    # 1-partition scalar math runs half as many, 2x-wider ops
                    if t % 2 == 0:
                        dot_ps = pmv.tile([1, 2, TILE], F32, tag="dot")
                        ss_ps = pmv.tile([1, 2, TILE], F32, tag="ss")
                        mv_ps["dot"] = dot_ps
                        mv_ps["ss"] = ss_ps
                    dot = mv_ps["dot"][:, t % 2, :]
                    ss = mv_ps["ss"][:, t % 2, :]
                    nc.tensor.matmul(dot, wss[:, 0:1], xs[0], start=True, stop=False)
                    nc.tensor.matmul(ss, ones_128, xqs[0], start=True, stop=False)
                    nc.tensor.matmul(dot, wss[:, 1:2], xs[1], start=False, stop=True)
                    nc.tensor.matmul(ss, ones_128, xqs[1], start=False, stop=True)
                    if t % 2 == 0:
                        return
                    # out = dot / sqrt(ss + 1e-24) + bs  (on the 2-tile pair)
                    dot2 = mv_ps["dot"].rearrange("p a r -> p (a r)")
                    ss2 = mv_ps["ss"].rearrange("p a r -> p (a r)")
                    rt = epp.tile([1, 2 * TILE], F32, tag="rt")
                    nc.scalar.activation(rt, ss2, AF.Sqrt, bias=eps_sb[0:1, 0:1])
                    rc = epp.tile([1, 2 * TILE], F32, tag="rc")
                    nc.vector.reciprocal_approx_fast(out=rc, in_=rt)
                    tm = epp.tile([1, 2 * TILE], F32, tag="tm")
                    nc.vector.tensor_mul(tm, dot2, rc)
                    nc.vector.tensor_scalar(
                        out=out_sb[:, t - 1 : t + 1, :].rearrange("p a r -> p (a r)"),
                        in0=tm,
                        scalar1=bss, scalar2=None,
                        op0=mybir.AluOpType.add,
                    )
                    if t == NT // 2 - 1:
                        # first half of the output: overlap store with compute
                        nc.sync.dma_start(
                            out=out_d[:, 0 : R // 2],
                            in_=out_sb[:, 0 : NT // 2, :].rearrange(
                                "p t r -> p (t r)"
                            ),
                        )

                pending = []
                xtv2 = xt_d[0:256, :].rearrange(
                    "(c p) (bb r) -> p c bb r", c=2, bb=BPC
                )
                for bb in range(BPC):
                    # one batch = 1024 rows = 2 row-tiles; load x^T per batch
                    # (rows 0-255 arrive as one strided DMA into [128, 2, N])
                    x01 = xp.tile([128, 2, N], F16, tag="x01")
                    x2 = xp.tile([64, N], F16, tag="x2")
                    nc.sync.dma_start(out=x01, in_=xtv2[:, :, bb])
                    nc.sync.dma_start(out=x2, in_=xtv[256:320, bb])
                    x0 = x01[:, 0, :]
                    x1 = x01[:, 1, :]
                    for tt in range(2):
                        t = bb * 2 + tt
                        rs = slice(tt * TILE, (tt + 1) * TILE)
                        # L1 ordered so ph0 finishes 3 matmuls early: the h0
                        # relu and the first L2 matmuls overlap ph1's tail
                        ph0 = php.tile([128, TILE], F32, tag="ph0")
                        ph1 = php.tile([128, TILE], F32, tag="ph1")
                        nc.tensor.matmul(
                            ph0, w3t[:, 0, 0:128], x0[:, rs], start=True, stop=False
                        )
                        nc.tensor.matmul(
                            ph0, w3t[:, 1, 0:128], x1[:, rs], start=False, stop=False
                        )
                        # the two K=32 tail matmuls sit in different PE
                        # row-groups (rows 0-31 / 32-63) and run concurrently
                        nc.tensor.matmul(
                            ph0, w3c2[0:32, :], x2[0:32, rs], start=False, stop=True
                        )
                        nc.tensor.matmul(
                            ph1, w3c2[32:64, :], x2[32:64, rs], start=True, stop=False
                        )
                        nc.tensor.matmul(
                            ph1, w3t[:, 0, 128:256], x0[:, rs], start=False, stop=False
                        )
                        nc.tensor.matmul(
                            ph1, w3t[:, 1, 128:256], x1[:, rs], start=False, stop=True
                        )
                        # h0 on ACT; h1 on DVE (relu via op1=max) to balance load
                        h0 = wp.tile([128, TILE], F16, tag="h0")
                        nc.scalar.activation(
                            h0, ph0, AF.Relu, bias=b3s[:, 0:1], scale=1.0
                        )
                        h1 = wp.tile([128, TILE], F16, tag="h1")
                        nc.vector.tensor_scalar(
                            out=h1, in0=ph1,
                            scalar1=b3s[:, 1:2], scalar2=0.0,
                            op0=mybir.AluOpType.add, op1=mybir.AluOpType.max,
                        )
                        # L2 ordered h0-first so it can start before h1 is done
                        po0 = pop.tile([128, TILE], F32, tag="po0")
                        po1 = pop.tile([128, TILE], F32, tag="po1")
                        nc.tensor.matmul(
                            po0, w4t[:, 0, 0:128], h0, start=True, stop=False
                        )
                        nc.tensor.matmul(
                            po1, w4t[:, 0, 128:256], h0, start=True, stop=False
                        )
                        nc.tensor.matmul(
                            po0, w4t[:, 1, 0:128], h1, start=False, stop=True
                        )
                        nc.tensor.matmul(
                            po1, w4t[:, 1, 128:256], h1, start=False, stop=True
                        )
                        xvt = wp.tile([128, 2, TILE], F16, tag="xv")
                        for o, po in ((0, po0), (1, po1)):
                            nc.scalar.activation(
                                xvt[:, o, :], po, AF.Relu,
                                bias=b4af[:, o, bb : bb + 1],
                                scale=af[:, o, bb : bb + 1],
                            )
                        xqt = wp.tile([128, 2, TILE], F16, tag="xq")
                        nc.gpsimd.tensor_mul(xqt, xvt, xvt)
                        if t == 0:
                            emit_attention_b()
                        # emit matvecs at a 2-tile lag: keeps the in-order PE
                        # queue from stalling on ACT xv -> GpSimd xq latency
                        pending.append((t, [xvt[:, 0, :], xvt[:, 1, :]],
                                        [xqt[:, 0, :], xqt[:, 1, :]]))
                        if len(pending) > 2:
                            emit_mv_epilogue(*pending.pop(0))
                for p in pending:
                    emit_mv_epilogue(*p)
                nc.sync.dma_start(
                    out=out_d[:, R // 2 : R],
                    in_=out_sb[:, NT // 2 : NT, :].rearrange("p t r -> p (t r)"),
                )
    nc.compile()
    return nc


_NC_CACHE = {}


def _get_nc():
    if "nc" not in _NC_CACHE:
        _NC_CACHE["nc"] = _build_nc()
    return _NC_CACHE["nc"]


def _f16(x):
    return np.ascontiguousarray(x).astype(np.float16)


def kernel(**inputs) -> np.ndarray:
    object_feat = np.ascontiguousarray(np.asarray(inputs["object_feat"], np.float32))
    lang_feat = np.ascontiguousarray(np.asarray(inputs["lang_feat"], np.float32))
    lang_mask = np.asarray(inputs["lang_mask"])
    W3 = np.asarray(inputs["W3"], np.float32)
    W4 = np.asarray(inputs["W4"], np.float32)
    b3 = np.asarray(inputs["b3"], np.float32)
    b4 = np.asarray(inputs["b4"], np.float32)
    Wa = np.asarray(inputs["Wa"], np.float32)
    Ws = np.asarray(inputs["Ws"], np.float32)
    bs = np.asarray(inputs["bs"], np.float32)

    w3r = _f16(W3[0:256].reshape(2, 128, D).transpose(1, 0, 2).reshape(128, 2 * D))
    w3c2 = np.concatenate([W3[256:288, 0:128], W3[256:288, 128:256]], axis=0)
    w3cr = _f16(w3c2)
    w4r = _f16(W4.reshape(2, 128, D).transpose(1, 0, 2).reshape(128, 2 * D))
    csth = np.zeros((128, 4), np.float16)
    csth[:, 0:2] = _f16(Ws[:, 0].reshape(2, 128).T)
    csth[:, 2:4] = _f16(Wa[D:, 0].reshape(2, 128).T)

    in_maps = []
    for i in range(NCORES):
        sl = slice(i * BPC, (i + 1) * BPC)
        of = object_feat[sl]                                   # (BPC, N, C1)
        lf = lang_feat[sl]                                     # (BPC, M, D)
        xt = of.reshape(R, C1).T
        xt_dup = np.concatenate([xt, xt[256:288]], axis=0)     # (320, R)
        cstf = np.zeros((128, 13), np.float32)
        cstf[:, 0:2] = b3.reshape(2, 128).T
        cstf[:, 2:4] = b4.reshape(2, 128).T
        cstf[0, 4] = bs[0]
        cstf[0:M, 5:13] = lang_mask[sl].T.astype(np.float32)
        lt2 = lf.transpose(2, 0, 1).reshape(2, 128, BPC * M)
        in_maps.append(
            {
                "xt": _f16(xt_dup),
                "langm": _f16(lf.transpose(1, 0, 2)),
                "langt2": _f16(lt2.transpose(1, 0, 2).reshape(128, 2 * BPC * M)),
                "cstf": cstf,
                "csth": csth,
                "w3": w3r,
                "w3c": w3cr,
                "w4": w4r,
            }
        )

    nc = _get_nc()
    res = run_bass_kernel_spmd(nc, in_maps, core_ids=list(range(NCORES)))
    _NC_CACHE["last_results"] = res
    out = np.empty((B, 1, N), np.float32)
    for i in range(NCORES):
        out[i * BPC : (i + 1) * BPC, 0, :] = res.results[i]["out"].reshape(BPC, N)
    return out
